# revision 1
# baseline (speedup 1.0000x reference)
"""Differentiable point-cloud renderer (bilinear splat) as a Bass/Tile kernel
for 8 Trainium2 NeuronCores.

Formulation: the bilinear scatter-add of point n into image[y, x] factorizes
as an outer product of 1-D "hat" functions:

    image[y, x] = sum_n featm_n * hat(y - py_n) * hat(x - px_n)
    hat(t) = relu(1 - |t|)

so per batch the image is a single matmul  image = A^T @ B  with
    A[n, y] = featm_n * hat(y - py_n)   (lhsT, fp16)
    B[n, x] = hat(x - px_n)             (rhs,  fp16)
contracting over points in K-tiles of 128 on the PE, accumulating in PSUM.

Sharding: pure data parallel, 16 batches per core. The 3 identical output
channels are replicated on the host (identical data).
"""

import functools
import sys

sys.path.insert(0, "/opt/trn_rl_repo")

import numpy as np

import concourse.bacc as bacc
import concourse.bass as bass
import concourse.mybir as mybir
import concourse.tile as tile
from concourse.bass_utils import run_bass_kernel_spmd
from concourse.masks import make_identity

from concourse import dve_ops as _dve_ops
from concourse.dve_spec import (
    C0 as _C0, C1 as _C1, C2 as _C2, Spec as _Spec, Src0 as _Src0,
    Zero as _Zero, lower as _dve_lower, maxx as _maxx, minn as _minn,
)
from concourse.dve_uop import DveOpSpec as _DveOpSpec


def _register_neghat():
    """Custom fused DVE op: out = min(|in0 - s0| + imm2, 0) * s1.
    With in0 = iota, s0 = p, s1 = f, imm2 = -1 this is -f*hat(j - p)
    in a single 1x DVE instruction."""
    for o in _dve_ops.OPS:
        if o.name == "NEGHAT_ANT":
            return o
    d = _Src0 - _C0
    spec = _Spec(
        body=_minn(_maxx(d, _Zero - d) + _C2, _Zero) * _C1,
        reference=lambda in0, in1, s0, s1, imm2: (
            np.minimum(np.abs(in0.astype(np.float32) - s0) + imm2, 0.0) * s1
        ).astype(np.float32),
    )
    row = _dve_ops._CUSTOM_DVE_ROW_BASE + len(_dve_ops.OPS)
    assert row < 0x20
    op = _dve_ops.DveOp("NEGHAT_ANT", spec, subdim=False, uops_sha={})
    for ver in ("v3", "v4"):
        try:
            u = _dve_lower(spec, ver=ver)
            op.uops_sha[ver] = _DveOpSpec(
                name="NEGHAT_ANT", opcode=row, uops=u, rd1_en=False
            ).sha(ver)
        except Exception:
            pass
    _dve_ops.OPS.append(op)
    _dve_ops._SUB_OPCODE_FOR_NAME["NEGHAT_ANT"] = row
    _dve_ops.CUSTOM_DVE_SPECS["NEGHAT_ANT"] = spec
    return op


NEGHAT = _register_neghat()


def _register_neghat_post():
    """out = min(in0 + imm2, 0) * s1 — 3-stage finisher (2x-eligible)."""
    for o in _dve_ops.OPS:
        if o.name == "NEGHATP_ANT":
            return o
    spec = _Spec(
        body=_minn(_Src0 + _C2, _Zero) * _C1,
        reference=lambda in0, in1, s0, s1, imm2: (
            np.minimum(in0.astype(np.float32) + imm2, 0.0) * s1
        ).astype(np.float32),
    )
    row = _dve_ops._CUSTOM_DVE_ROW_BASE + len(_dve_ops.OPS)
    assert row < 0x20
    op = _dve_ops.DveOp("NEGHATP_ANT", spec, subdim=False, uops_sha={},
                        perf_en={"v3": True, "v4": True})
    for ver in ("v3", "v4"):
        try:
            u = _dve_lower(spec, ver=ver)
            op.uops_sha[ver] = _DveOpSpec(
                name="NEGHATP_ANT", opcode=row, uops=u, rd1_en=False
            ).sha(ver)
        except Exception:
            pass
    _dve_ops.OPS.append(op)
    _dve_ops._SUB_OPCODE_FOR_NAME["NEGHATP_ANT"] = row
    _dve_ops.CUSTOM_DVE_SPECS["NEGHATP_ANT"] = spec
    return op


NEGHATP = _register_neghat_post()

B, N, H, W = 128, 16384, 224, 224
NCORES = 8
BPC = B // NCORES            # batches per core
KT = N // 128                # k-tiles (of 128 points) per batch
F32 = mybir.dt.float32
F16 = mybir.dt.float16
I32 = mybir.dt.int32
AF = mybir.ActivationFunctionType
OP = mybir.AluOpType
AX = mybir.AxisListType
HPI = float(np.pi / 2)


def splat_kernel(tc, nc, pts_d, az_d, el_d, img_d):
    act = nc.scalar.activation
    ts_ = nc.vector.tensor_scalar
    tt_ = nc.vector.tensor_tensor
    stt = nc.vector.scalar_tensor_tensor

    with (
        tc.tile_pool(name="const", bufs=1) as cpool,
        tc.tile_pool(name="persist", bufs=1) as ppool,
        tc.tile_pool(name="work", bufs=3) as wpool,
        tc.tile_pool(name="hat", bufs=4) as hpool,
        tc.tile_pool(name="psum", bufs=2, space="PSUM") as pspool,
        tc.tile_pool(name="psmall", bufs=1, space="PSUM") as pspool2,
    ):
        # ---------------- constants ----------------
        ident = cpool.tile([128, 128], F32)
        make_identity(nc, ident[:])
        iota_i = cpool.tile([128, W], I32)
        nc.gpsimd.iota(iota_i[:], pattern=[[1, W]], base=0, channel_multiplier=0)
        iota_f = cpool.tile([128, W], F32)
        nc.vector.tensor_copy(iota_f[:], iota_i[:])
        iota_h = cpool.tile([128, W], F16)
        nc.vector.tensor_copy(iota_h[:], iota_f[:])
        ones_row = cpool.tile([1, 128], F32)
        nc.vector.memset(ones_row[:], 1.0)

        # ---------------- rotation coefficients ----------------
        # R = R_el @ R_az ;  rx = x*ca + z*sa
        #                    ry = x*(se*sa) + y*ce + z*(-se*ca)
        #                    rz = x*(-ce*sa) + y*se + z*(ce*ca)
        az_sb = cpool.tile([1, BPC], F32)
        nc.sync.dma_start(out=az_sb[:], in_=az_d[None, :])
        el_sb = cpool.tile([1, BPC], F32)
        nc.sync.dma_start(out=el_sb[:], in_=el_d[None, :])
        Rrow = cpool.tile([1, 8 * BPC], F32)
        hpi = cpool.tile([1, 1], F32)
        nc.vector.memset(hpi[:], HPI)
        zero1 = cpool.tile([1, 1], F32)
        nc.vector.memset(zero1[:], 0.0)

        def sl(k):
            return Rrow[:, k * BPC:(k + 1) * BPC]

        # ScalarE Sin is only valid on [-pi, pi]; range-reduce args first.
        TPI = float(2 * np.pi)

        def sin_wrapped(out_ap, in_ap, shift):
            c = cpool.tile([1, BPC], F32, tag="sinw_c")
            if shift != 0.0:
                ts_(c[:], in_ap, shift, None, OP.add)
            else:
                nc.vector.tensor_copy(c[:], in_ap)
            m = cpool.tile([1, BPC], F32, tag="sinw_m")
            ts_(m[:], c[:], float(np.pi), None, OP.is_ge)
            w = cpool.tile([1, BPC], F32, tag="sinw_w")
            stt(w[:], m[:], -TPI, c[:], op0=OP.mult, op1=OP.add)
            act(out_ap, w[:], AF.Sin, bias=zero1[:])

        sin_wrapped(sl(0), az_sb[:], HPI)   # ca
        sin_wrapped(sl(1), az_sb[:], 0.0)   # sa
        sin_wrapped(sl(3), el_sb[:], HPI)   # ce
        sin_wrapped(sl(6), el_sb[:], 0.0)   # se
        tt_(sl(2), sl(6), sl(1), op=OP.mult)                      # se*sa
        stt(sl(4), sl(6), -1.0, sl(0), op0=OP.mult, op1=OP.mult)  # -se*ca
        stt(sl(5), sl(3), -1.0, sl(1), op0=OP.mult, op1=OP.mult)  # -ce*sa
        tt_(sl(7), sl(3), sl(0), op=OP.mult)                      # ce*ca

        # broadcast R coeffs to all 128 partitions via ones-matmul
        Rp = pspool2.tile([128, 8 * BPC], F32, tag='ptmp')
        nc.tensor.matmul(out=Rp[:], lhsT=ones_row[:], rhs=Rrow[:],
                         start=True, stop=True)
        Rbc = cpool.tile([128, 8 * BPC], F32)
        nc.vector.tensor_copy(Rbc[:], Rp[:])

        def Rc(k, b):
            return Rbc[:, k * BPC + b:k * BPC + b + 1]

        # ---------------- phase 1: coordinates per batch ----------------
        # Layout: point index n = p*128 + q; partition p, k-tile q.
        # pxE = px + 0.5 = (rx+1)*112 ; pyE likewise.
        px_all = ppool.tile([128, BPC * 128], F32)
        py_all = ppool.tile([128, BPC * 128], F32)
        rz_all = ppool.tile([128, BPC * 128], F32)
        # min in cols [0:BPC], max in cols [32:32+BPC] (32-aligned partition
        # bases after the transpose)
        zred = ppool.tile([128, 64], F32)
        nc.vector.memset(zred[:], 0.0)

        for b in range(BPC):
            pts = wpool.tile([128, 384], F32)
            nc.sync.dma_start(
                out=pts[:],
                in_=pts_d[b].rearrange("(p q) c -> p (q c)", p=128),
            )
            pv = pts[:].rearrange("p (q c) -> p c q", c=3)
            x, y, z = pv[:, 0, :], pv[:, 1, :], pv[:, 2, :]

            pxb = px_all[:, b * 128:(b + 1) * 128]
            pyb = py_all[:, b * 128:(b + 1) * 128]
            rzb = rz_all[:, b * 128:(b + 1) * 128]

            t1 = wpool.tile([128, 128], F32)
            ts_(t1[:], x, Rc(0, b), None, OP.mult)
            rx = wpool.tile([128, 128], F32)
            stt(rx[:], z, Rc(1, b), t1[:], op0=OP.mult, op1=OP.add)
            ts_(pxb, rx[:], 1.0, 112.0, OP.add, OP.mult)

            t2 = wpool.tile([128, 128], F32)
            ts_(t2[:], x, Rc(2, b), None, OP.mult)
            t3 = wpool.tile([128, 128], F32)
            stt(t3[:], y, Rc(3, b), t2[:], op0=OP.mult, op1=OP.add)
            ry = wpool.tile([128, 128], F32)
            stt(ry[:], z, Rc(4, b), t3[:], op0=OP.mult, op1=OP.add)
            ts_(pyb, ry[:], 1.0, 112.0, OP.add, OP.mult)

            t4 = wpool.tile([128, 128], F32)
            ts_(t4[:], x, Rc(5, b), None, OP.mult)
            t5 = wpool.tile([128, 128], F32)
            stt(t5[:], y, Rc(6, b), t4[:], op0=OP.mult, op1=OP.add)
            stt(rzb, z, Rc(7, b), t5[:], op0=OP.mult, op1=OP.add)

            nc.vector.tensor_reduce(zred[:, b:b + 1], rzb, axis=AX.X, op=OP.min)
            nc.vector.tensor_reduce(zred[:, 32 + b:32 + b + 1], rzb,
                                    axis=AX.X, op=OP.max)

        # ---------------- phase 1b: z min/max across partitions ----------------
        ztp = pspool2.tile([64, 128], F32, tag='ptmp')
        nc.tensor.transpose(out=ztp[:], in_=zred[:], identity=ident[:])
        zmm = cpool.tile([64, 1], F32)
        nc.vector.memset(zmm[:], 0.0)
        nc.vector.tensor_reduce(zmm[0:BPC, :], ztp[0:BPC, :], axis=AX.X, op=OP.min)
        nc.vector.tensor_reduce(zmm[32:32 + BPC, :], ztp[32:32 + BPC, :],
                                axis=AX.X, op=OP.max)
        zrp = pspool2.tile([1, 64], F32, tag='ptmp')
        nc.tensor.transpose(out=zrp[:], in_=zmm[:],
                            identity=ident[0:64, 0:64])
        zrow = cpool.tile([1, 64], F32)
        nc.vector.tensor_copy(zrow[:], zrp[:])
        zbp = pspool2.tile([128, 64], F32, tag='ptmp')
        nc.tensor.matmul(out=zbp[:], lhsT=ones_row[:], rhs=zrow[:],
                         start=True, stop=True)
        zbc = cpool.tile([128, 64], F32)
        nc.vector.tensor_copy(zbc[:], zbp[:])

        # feat = 0.3 + 0.7*(z - zmin)/(zmax - zmin + 1e-6) = z*inv07 + beta
        d_ = cpool.tile([128, BPC], F32)
        stt(d_[:], zbc[:, 32:32 + BPC], 1e-6, zbc[:, 0:BPC],
            op0=OP.add, op1=OP.subtract)
        rec = cpool.tile([128, BPC], F32)
        nc.vector.reciprocal(rec[:], d_[:])
        inv07 = cpool.tile([128, BPC], F32)
        ts_(inv07[:], rec[:], 0.7, None, OP.mult)
        tb = cpool.tile([128, BPC], F32)
        tt_(tb[:], zbc[:, 0:BPC], inv07[:], op=OP.mult)
        beta = cpool.tile([128, BPC], F32)
        ts_(beta[:], tb[:], -1.0, 0.3, OP.mult, OP.add)

        # ---------------- phase 2: hats + matmul per batch ----------------
        # Negation trick: build Atn = -f*hat_y and Btn = -hat_x; the two
        # negations cancel in the matmul, so no fixup is needed.
        #   y-side: u' = ACT Abs(j*f - f*py) = f*|j-py| ;
        #           Atn = min(u'-f, 0) = -f*hat_y              (1 ACT + 1 DVE)
        #   x-side (DVE path): pn = min(j-1-px, 0), qn = min(px-1-j, 0)
        #           (2-src ts from shifted iota consts);
        #           Btn = max(pn, qn) = -hat_x  (8-tile-wide TT max)
        #   x-side (ACT path, to balance engines): ux = ACT Abs(j - px);
        #           Btn = min(ux-1, 0) (imm-chain)
        TW = 8                    # tiles per wide group
        NYACT = 0                 # y-tiles on the ACT path per batch
        BF16 = mybir.dt.bfloat16
        for b in range(BPC):
            pxE = px_all[:, b * 128:(b + 1) * 128]   # px + 0.5
            pyE = py_all[:, b * 128:(b + 1) * 128]
            rzb = rz_all[:, b * 128:(b + 1) * 128]

            feat = wpool.tile([128, 128], F32)
            ts_(feat[:], rzb, inv07[:, b:b + 1], beta[:, b:b + 1],
                OP.mult, OP.add)
            # mask: px>=0 & px<223 & py>=0 & py<223   (pxE = px+0.5)
            mx = wpool.tile([128, 128], F32)
            ts_(mx[:], pxE, 0.5, None, OP.is_ge)
            mx2 = wpool.tile([128, 128], F32)
            stt(mx2[:], pxE, 223.5, mx[:], op0=OP.is_lt, op1=OP.mult)
            my = wpool.tile([128, 128], F32)
            ts_(my[:], pyE, 0.5, None, OP.is_ge)
            my2 = wpool.tile([128, 128], F32)
            stt(my2[:], pyE, 223.5, my[:], op0=OP.is_lt, op1=OP.mult)
            fm = wpool.tile([128, 128], F32)
            tt_(fm[:], feat[:], mx2[:], op=OP.mult)
            featm = wpool.tile([128, 128], F32)
            tt_(featm[:], fm[:], my2[:], op=OP.mult)
            pym05 = wpool.tile([128, 128], F32)   # py
            ts_(pym05[:], pyE, 0.5, 0.0, OP.subtract, OP.add)
            pyneg = wpool.tile([128, 128], F32)   # -py
            ts_(pyneg[:], pyE, -1.0, 0.5, OP.mult, OP.add)
            pxneg = wpool.tile([128, 128], F32)   # -px
            ts_(pxneg[:], pxE, -1.0, 0.5, OP.mult, OP.add)

            ps0 = pspool.tile([128, W], F32)
            ps1 = pspool.tile([128, W], F32)

            def ymm(q, btn_ap):
                At = hpool.tile([128, W], F16, tag="At")
                if q < KT - NYACT:
                    nc.vector._custom_dve(
                        NEGHAT, out=At[:], in0=iota_h[:],
                        s0=pym05[:, q:q + 1], s1=featm[:, q:q + 1], imm2=-1.0)
                else:
                    uy = hpool.tile([128, W], F16, tag="uy")
                    act(uy[:], iota_f[:], AF.Abs, bias=pyneg[:, q:q + 1])
                    nc.vector._custom_dve(
                        NEGHATP, out=At[:], in0=uy[:],
                        s0=0.0, s1=featm[:, q:q + 1], imm2=-1.0)
                nc.tensor.matmul(out=ps0[:], lhsT=At[:, 0:128], rhs=btn_ap,
                                 start=(q == 0), stop=(q == KT - 1))
                nc.tensor.matmul(out=ps1[0:96, :], lhsT=At[:, 128:224],
                                 rhs=btn_ap, start=(q == 0), stop=(q == KT - 1))

            for g in range(KT // TW):
                q0 = g * TW
                uxw = hpool.tile([128, TW * W], F16, tag="uxw")
                for j in range(TW):
                    act(uxw[:, j * W:(j + 1) * W], iota_f[:], AF.Abs,
                        bias=pxneg[:, q0 + j:q0 + j + 1])
                btnw = hpool.tile([128, TW * W], F16, tag="btnw")
                ts_(btnw[:], uxw[:], 1.0, 0.0, OP.subtract, OP.min)
                for j in range(TW):
                    ymm(q0 + j, btnw[:, j * W:(j + 1) * W])

            out0 = wpool.tile([128, W], F32)
            nc.vector.tensor_copy(out0[:], ps0[:])
            out1 = wpool.tile([128, W], F32)
            nc.vector.tensor_copy(out1[0:96, :], ps1[0:96, :])
            nc.sync.dma_start(out=img_d[b, 0:128, :], in_=out0[:])
            nc.sync.dma_start(out=img_d[b, 128:224, :], in_=out1[0:96, :])


@functools.lru_cache(maxsize=1)
def _get_compiled():
    nc = bacc.Bacc(
        "TRN2",
        target_bir_lowering=False,
        debug=False,
        enable_asserts=False,
        num_devices=NCORES,
    )
    pts_d = nc.dram_tensor("points", [BPC, N, 3], F32, kind="ExternalInput")
    az_d = nc.dram_tensor("azimuth", [BPC], F32, kind="ExternalInput")
    el_d = nc.dram_tensor("elevation", [BPC], F32, kind="ExternalInput")
    img_d = nc.dram_tensor("img", [BPC, H, W], F32, kind="ExternalOutput")
    with tile.TileContext(nc) as tc:
        splat_kernel(tc, nc, pts_d, az_d, el_d, img_d)
    nc.compile()
    return nc


def run_on_device(points, azimuth, elevation, trace=False, **kw):
    nc = _get_compiled()
    in_maps = []
    for i in range(NCORES):
        s = slice(i * BPC, (i + 1) * BPC)
        in_maps.append({
            "points": np.ascontiguousarray(points[s], dtype=np.float32),
            "azimuth": np.ascontiguousarray(azimuth[s], dtype=np.float32),
            "elevation": np.ascontiguousarray(elevation[s], dtype=np.float32),
        })
    return run_bass_kernel_spmd(nc, in_maps, list(range(NCORES)),
                                trace=trace, **kw)


def kernel(points, azimuth, elevation):
    res = run_on_device(points, azimuth, elevation)
    imgs = np.concatenate([res.results[i]["img"] for i in range(NCORES)], axis=0)
    out = np.empty((B, 3, H, W), dtype=np.float32)
    out[:] = imgs[:, None, :, :]
    return out

